# revision 34
# baseline (speedup 1.0000x reference)
"""Bass/Trainium2 kernel for batched dot-product attention.

Problem: q,k,v [B=4, S=4096, D=1024]; projections to dk=dv=128; softmax
attention per batch element.  Sharded over 8 NeuronCores as (batch,
query-half): core c handles batch c//2, queries (c%2)*2048 ... +2048.

All layouts on-chip keep the contraction dimension on SBUF partitions:
  qT/kT     [d_model, seq]    (host pre-transposed, fp8 pair-interleaved)
  vT        [d_model, seq]    (host pre-transposed, bf16)
  kpT/qpT   [dk, seq]         (projection output, bf16)
  vp        [seq, dv]         (via DMA crossbar transpose, bf16)
  S^T tiles [keys, q]         (scores transposed, f32 PSUM)
  out^T     [dv, q]           (bf16; host divides by sums and undoes)

Q/K projections run as fp8e4 DoubleRow matmuls (contraction 256/step,
2 fp8 per cycle -- requires the moving operand pair-interleaved in
memory and the stationary in [p, 2, d] block layout).  wq stays
UNSCALED so it avoids fp8 subnormals; the 1/sqrt(dk) folds into the
exp's scale operand instead.  V stays bf16 for accuracy.

Query blocks are processed in PAIRS (1024-wide exp tiles amortize the
ScalarE per-op overhead and halve AV weight loads).  Pair 0's attention
chunks interleave with the projection kb-loop; pair 1 defers its AV
matmuls (exp tiles parked in SBUF) until PSUM banks free up, bursting
per query-half at the end so output copies overlap remaining matmuls.
vp transposes ride the sync-queue DMA crossbar, pipelined two blocks
ahead (psv+cast one iteration before the transpose issue) so the sync
engine never stalls the input stream.  Softmax denominators (sum over
keys = partition axis) via a ones-vector matmul over a bf16 pairwise
accumulation tree; normalization on the host.

Bias algebra: bk drops entirely (a per-query score shift is softmax-
invariant); bv is applied on the host (softmax weights sum to 1); bq
adds to qpT on-chip.  Weights ship as one packed bf16 tensor (wv, bq,
ones) plus one fp8 tensor (wq, wk), fetched first on the sync queue.
A burst of dummy matmuls on scratch SBUF warms the PE clock (HAM)
before real data lands.
"""

import math

import numpy as np
import ml_dtypes

import concourse.bass as bass
import concourse.tile as tile
from concourse import bacc, mybir
from concourse.bass_utils import run_bass_kernel_spmd

B, S, DM, DK, DV = 4, 4096, 1024, 128, 128
N_CORES = 8
SQ = S // 2          # queries per core
NQB = SQ // 512      # query blocks of 512 per core (4)
NKC = S // 128       # key chunks of 128 (32)
NMC = DM // 128      # d_model chunks (8)
NKB = S // 512       # key blocks of 512 (8)

BF16 = mybir.dt.bfloat16
F32 = mybir.dt.float32
FP8 = mybir.dt.float8e4
NP_BF16 = ml_dtypes.bfloat16
NP_FP8 = ml_dtypes.float8_e4m3
DoubleRow = mybir.MatmulPerfMode.DoubleRow

E_DT = BF16          # dtype of exp tiles (AV moving operand)
ACC_DT = BF16        # dtype of the denominator accumulation tree
AV_STAGGER = 2       # pair-chunks the exp/AV drain lags the S matmuls
N_WARMUP = 10        # dummy matmuls to keep the PE HAM-warm at startup

Identity = mybir.ActivationFunctionType.Identity
Copy = mybir.ActivationFunctionType.Copy
Exp = mybir.ActivationFunctionType.Exp

# packed weight layout: columns [wv | bq | ones]; wq/wk ship as fp8
WCOL_V = 0
WCOL_BQ, WCOL_ONES = DM, DM + 1
WCOLS = DM + 2
SCALE = 1.0 / math.sqrt(DK)


def _emit(tc: tile.TileContext, aps: dict):
    nc = tc.nc
    qT, kT, vT = aps["qT"], aps["kT"], aps["vT"]
    outT = aps["outT"]

    with tc.tile_pool(name="persist", bufs=1) as persist:
        # --- packed constants (one DMA, first on the sync queue) ---
        w_sb = persist.tile([128, WCOLS], BF16, tag="w_pack", name="w_pack")
        nc.sync.dma_start(w_sb[:], aps["w_pack"][:])

        def wslice(base, c):
            return w_sb[:, base + c * 128: base + (c + 1) * 128]

        # w8[:, 0] = wq (unscaled; 1/sqrt(dk) folds into the exp scale),
        # w8[:, 1] = wk; DoubleRow block layout [p, qk, c, j, d]
        w8_sb = persist.tile([128, 2, NMC // 2, 2, 128], FP8, tag="w8", name="w8")
        ones_ap = w_sb[:, WCOL_ONES:WCOL_ONES + 1]
        bq_f32 = persist.tile([128, 1], F32, tag="bq_f32", name="bq_f32")
        nc.vector.tensor_copy(bq_f32[:], w_sb[:, WCOL_BQ:WCOL_BQ + 1])
        bq_ap = bq_f32[:]

        # --- PE warm-up scratch (no data deps; HAM warms before real MMs) ---
        warm_sb = persist.tile([128, 512], BF16, tag="warm", name="warm_sb")

        # --- persistent activations ---
        kpT_blk = [persist.tile([128, 512], BF16, tag=f"kpT{i}", name=f"kpT{i}")
                   for i in range(NKB)]
        qpT_t = [persist.tile([128, 512], BF16, tag=f"qpT{i}", name=f"qpT{i}")
                 for i in range(NQB)]
        # vp per key-block: vp_blk[kb][p, j, :] = projected V row for key
        # 512*kb + 128*j + p (one xbar transpose per block)
        vp_blk = [persist.tile([128, 4, 128], BF16, tag=f"vpb{i}", name=f"vpb{i}")
                  for i in range(NKB)]
        sums_sb = persist.tile([1, SQ], F32, tag="sums", name="sums_sb")

        with (
            tc.tile_pool(name="op", bufs=2, space="PSUM") as op,
            tc.tile_pool(name="pp", bufs=2, space="PSUM") as pp,
            tc.tile_pool(name="sp", bufs=2, space="PSUM") as sp,
            tc.tile_pool(name="xs", bufs=2) as xs,
            tc.tile_pool(name="ep", bufs=6) as ep,
            tc.tile_pool(name="tp", bufs=3) as tp,
            tc.tile_pool(name="e1p", bufs=1) as e1p,
            tc.tile_pool(name="accp", bufs=2) as accp,
            tc.tile_pool(name="miscp", bufs=2) as miscp,
        ):
            # ---- PE warm-up: dummy matmuls on scratch, discarded ----
            nc.gpsimd.memset(warm_sb[:], 0.0)
            warm_ps = sp.tile([128, 1024], F32, tag="sp", name="warm_ps")
            for _ in range(N_WARMUP):
                nc.tensor.matmul(warm_ps[:, 0:512], lhsT=warm_sb[:, 0:128],
                                 rhs=warm_sb[:], start=True, stop=True)
                nc.tensor.matmul(warm_ps[:, 512:1024], lhsT=warm_sb[:, 0:128],
                                 rhs=warm_sb[:], start=True, stop=True)

            # ---- input fetch + qp projection helpers ----
            kxs, vxs = {}, {}

            def fetch_kx(kb):
                # [p, c, s, j]: dm = 256c + 128j + p, pair elements adjacent
                # so the DoubleRow matmul streams 2 fp8 per cycle
                kx = xs.tile([128, NMC // 2, 512, 2], FP8, tag="kx",
                             name=f"kx{kb}", bufs=4)
                nc.sync.dma_start(kx[:], kT[kb])
                kxs[kb] = kx

            def fetch_vx(kb):
                vx = xs.tile([128, NMC, 512], BF16, tag="vx", name=f"vx{kb}",
                             bufs=4)
                nc.sync.dma_start(vx[:], vT[kb])
                vxs[kb] = vx

            qxs = {}

            def fetch_q(qb):
                qx = xs.tile([128, NMC // 2, 512, 2], FP8, tag="qx",
                             name=f"qx{qb}", bufs=4)
                nc.sync.dma_start(qx[:], qT[qb])
                qxs[qb] = qx

            def project_q(qb):
                qx = qxs.pop(qb)
                psq = sp.tile([128, 512], F32, tag="sp", name=f"psq{qb}")
                for c in range(NMC // 2):
                    nc.tensor.matmul(
                        psq[:], lhsT=w8_sb[:, 0, c, :, :],
                        rhs=qx[:, c, :, :].rearrange("p n j -> p j n"),
                        start=(c == 0), stop=(c == NMC // 2 - 1),
                        perf_mode=DoubleRow,
                    )
                nc.vector.tensor_scalar_add(qpT_t[qb][:], psq[:], bq_ap)

            # stream order: weights already queued first; then the tensors
            # needed to unlock pair-0 attention (qx0, kx0, qx1), then vx0
            # for the AV chain, then the rest.
            nc.sync.dma_start(w8_sb[:], aps["w8"][:])
            fetch_q(0)
            fetch_q(1)
            fetch_kx(0)
            fetch_kx(1)
            fetch_vx(0)
            fetch_q(2)
            fetch_q(3)
            fetch_vx(1)

            def proj_k(kb):
                kx = kxs.pop(kb)
                psk = pp.tile([128, 512], F32, tag="pp", name=f"psk{kb}")
                for c in range(NMC // 2):
                    nc.tensor.matmul(
                        psk[:], lhsT=w8_sb[:, 1, c, :, :],
                        rhs=kx[:, c, :, :].rearrange("p n j -> p j n"),
                        start=(c == 0), stop=(c == NMC // 2 - 1),
                        perf_mode=DoubleRow,
                    )
                nc.vector.tensor_copy(kpT_blk[kb][:], psk[:])

            # ---- attention pair machinery ----
            def pair_begin(pidx, spool, defer_av=False):
                qa, qb_ = 2 * pidx, 2 * pidx + 1
                return dict(
                    p=pidx, qs=(qa, qb_), sp=spool, defer=defer_av,
                    o=None if defer_av else
                      [op.tile([128, 512], F32, tag="op", name=f"o{q}")
                       for q in (qa, qb_)],
                    acc=accp.tile([128, 1024], ACC_DT, tag="acc",
                                  name=f"acc{pidx}"),
                    pend=[], evs=[],
                )

            def pair_drain(st):
                kc, s = st["pend"].pop(0)
                if st["defer"]:
                    e = e1p.tile([128, 1024], E_DT, tag=f"e{st['p']}d_{kc}",
                                 name=f"e{st['p']}_{kc}")
                else:
                    e = ep.tile([128, 1024], E_DT, tag="e", name=f"e{st['p']}_{kc}")
                nc.scalar.activation(e[:], s[:], Exp, scale=SCALE)
                if kc % 2 == 0:
                    st["elast"] = e
                else:
                    # one bf16 add level halves the accumulate traffic; the
                    # acc tree stays bf16 for DVE 2x mode.  The serial
                    # acc-chain alternates onto GpSimd so DVE stays free for
                    # the projection casts (PSUM recycling path).
                    tmp = tp.tile([128, 1024], ACC_DT, tag="tmp",
                                  name=f"t{st['p']}_{kc}")
                    nc.vector.tensor_add(tmp[:], st["elast"][:], e[:])
                    if kc == 1:
                        nc.vector.tensor_copy(st["acc"][:], tmp[:])
                    else:
                        nc.vector.tensor_add(st["acc"][:], st["acc"][:], tmp[:])
                if st["defer"]:
                    st["evs"].append((kc, e))
                    return
                av_emit(st, kc, e)

            def av_emit(st, kc, e):
                vps = vp_blk[kc // 4][:, kc % 4, :]
                for h in range(2):
                    nc.tensor.matmul(
                        st["o"][h][:], lhsT=vps, rhs=e[:, h * 512:(h + 1) * 512],
                        start=(kc == 0), stop=(kc == NKC - 1),
                    )

            def pair_chunk(st, kc):
                s = st["sp"].tile([128, 1024], F32, tag="sp", name=f"s{st['p']}_{kc}")
                kslice = kpT_blk[kc // 4][:, (kc % 4) * 128:(kc % 4 + 1) * 128]
                for h in range(2):
                    nc.tensor.matmul(
                        s[:, h * 512:(h + 1) * 512], lhsT=kslice,
                        rhs=qpT_t[st["qs"][h]][:], start=True, stop=True,
                    )
                st["pend"].append((kc, s))
                if len(st["pend"]) > AV_STAGGER:
                    pair_drain(st)

            def pair_flush(st):
                while st["pend"]:
                    pair_drain(st)
                if st["defer"]:
                    # use the projection pool's banks (free after the kb
                    # loop) so the burst need not wait for pair 0's output
                    # copies to release the op slots
                    st["o"] = [pp.tile([128, 512], F32, tag="pp", name=f"o{q}")
                               for q in st["qs"]]
                    for kc, e in st["evs"]:
                        av_emit(st, kc, e)

            def pair_tail_half(st, h):
                q = st["qs"][h]
                ps_sum = st["sp"].tile([1, 512], F32, tag="sp", name=f"pssum{q}")
                nc.tensor.matmul(
                    ps_sum[:], lhsT=ones_ap,
                    rhs=st["acc"][:, h * 512:(h + 1) * 512],
                    start=True, stop=True,
                )
                nc.scalar.activation(
                    sums_sb[:, q * 512:(q + 1) * 512], ps_sum[:], Copy
                )
                outsb = miscp.tile([128, 512], BF16, tag="out", name=f"out{q}")
                nc.vector.tensor_copy(outsb[:], st["o"][h][:])
                nc.sync.dma_start(outT[:, q * 512:(q + 1) * 512], outsb[:])

            def pair_tail(st):
                for h in range(2):
                    pair_tail_half(st, h)

            vpts = {}

            def proj_v_mm(kb):
                # V-projection matmuls + PSUM->SBUF cast only; the xbar
                # transpose is issued separately, one iteration later, so
                # it never blocks the sync queue waiting on vpt
                vx = vxs.pop(kb)
                psv = pp.tile([128, 512], F32, tag="pp", name=f"psv{kb}")
                for c in range(NMC):
                    nc.tensor.matmul(
                        psv[:], lhsT=wslice(WCOL_V, c), rhs=vx[:, c, :],
                        start=(c == 0), stop=(c == NMC - 1),
                    )
                vpt = xs.tile([128, 512], BF16, tag="vpt", name=f"vpt{kb}",
                              bufs=3)
                nc.vector.tensor_copy(vpt[:], psv[:])
                vpts[kb] = vpt

            def proj_v_tr(kb):
                nc.sync.dma_start_transpose(vp_blk[kb][:], vpts.pop(kb)[:])

            # ---- kb loop: kp + vp projection, pair-0 attention interleaved ----
            st0 = pair_begin(0, sp)
            st1 = pair_begin(1, sp, defer_av=True)
            project_q(0)
            project_q(1)
            proj_k(0)
            for kb in range(NKB):
                if kb + 2 < NKB:
                    fetch_kx(kb + 2)
                    fetch_vx(kb + 2)

                if kb == 0:
                    # vp0 is not ready until vx0 lands, so pair 0's first
                    # AVs are deferred (exp-only) and burst in iteration 1
                    st0["defer"] = True
                    pair_chunk(st0, 0)
                    pair_chunk(st0, 1)
                    proj_k(1)
                    # vpt0's cast must precede the exp-gated tree ops on the
                    # DVE queue, so its transpose issues early and the sync
                    # engine never stalls iteration 1's input fetches
                    proj_v_mm(0)
                    proj_v_tr(0)
                    pair_chunk(st0, 2)
                    pair_chunk(st0, 3)
                    # drain fully: st1's S matmuls below recycle these PSUM
                    # slots, and their release must not depend on scalar-queue
                    # work emitted after st1's exps (circular wait)
                    while st0["pend"]:
                        pair_drain(st0)
                    project_q(2)
                    project_q(3)
                    for kc in range(4):
                        pair_chunk(st1, kc)
                    proj_v_mm(1)
                    proj_v_tr(1)
                    proj_v_mm(2)
                else:
                    # projections first: they have no PSUM/exp coupling, so
                    # the PE works while ScalarE catches up on the exp chain
                    if kb + 1 < NKB:
                        proj_k(kb + 1)
                        # transpose for block kb+1: its vpt was cast last
                        # iteration, so the sync engine never waits on it
                        proj_v_tr(kb + 1)
                    if kb == 1:
                        st0["defer"] = False
                        for kc_, e_ in st0["evs"]:
                            av_emit(st0, kc_, e_)
                        st0["evs"] = []
                    kc = 4 * kb
                    pair_chunk(st0, kc)
                    pair_chunk(st1, kc)
                    pair_chunk(st0, kc + 1)
                    pair_chunk(st1, kc + 1)
                    pair_chunk(st0, kc + 2)
                    pair_chunk(st1, kc + 2)
                    if kb + 2 < NKB:
                        proj_v_mm(kb + 2)
                    pair_chunk(st0, kc + 3)
                    pair_chunk(st1, kc + 3)
            # early AV burst: everything already exp'd can run on the PE
            # while ScalarE finishes the exp-chain tail
            st1["o"] = [pp.tile([128, 512], F32, tag="pp", name=f"o{q}")
                        for q in st1["qs"]]
            for kc, e in st1["evs"]:
                av_emit(st1, kc, e)
            st1["evs"] = []
            pair_flush(st0)
            pair_tail(st0)
            while st1["pend"]:
                pair_drain(st1)
            # final chunks per half: half 0 finishes first so its output
            # copy + DMA overlap half 1's remaining AV matmuls
            for h in range(2):
                for kc, e in st1["evs"]:
                    nc.tensor.matmul(
                        st1["o"][h][:], lhsT=vp_blk[kc // 4][:, kc % 4, :],
                        rhs=e[:, h * 512:(h + 1) * 512],
                        start=(kc == 0), stop=(kc == NKC - 1),
                    )
                pair_tail_half(st1, h)
            nc.sync.dma_start(aps["sums"][:], sums_sb[:])


_CACHE = {}


def _build():
    if "nc" in _CACHE:
        return _CACHE["nc"]
    nc = bacc.Bacc("TRN2", debug=False, num_devices=N_CORES)
    aps = {
        "qT": nc.dram_tensor("qT", [NQB, 128, NMC // 2, 512, 2], FP8,
                             kind="ExternalInput").ap(),
        "kT": nc.dram_tensor("kT", [NKB, 128, NMC // 2, 512, 2], FP8,
                             kind="ExternalInput").ap(),
        "vT": nc.dram_tensor("vT", [NKB, 128, NMC, 512], BF16,
                             kind="ExternalInput").ap(),
        "w_pack": nc.dram_tensor("w_pack", [128, WCOLS], BF16,
                                 kind="ExternalInput").ap(),
        "w8": nc.dram_tensor("w8", [128, 2, NMC // 2, 2, 128], FP8,
                             kind="ExternalInput").ap(),
        "outT": nc.dram_tensor("outT", [DV, SQ], BF16, kind="ExternalOutput").ap(),
        "sums": nc.dram_tensor("sums", [1, SQ], F32, kind="ExternalOutput").ap(),
    }
    with tile.TileContext(nc) as tc:
        _emit(tc, aps)
    nc.compile()
    _CACHE["nc"] = nc
    return nc


def _pack_x(xT, nblk):
    # [DM, n] -> [nblk, 128, NMC, 512]  (contiguous per-stripe layout)
    return np.ascontiguousarray(
        xT.reshape(NMC, 128, nblk, 512).transpose(2, 1, 0, 3))


def make_in_maps(q, k, v, wq, bq, wk, bk, wv, bv):
    scale = 1.0 / math.sqrt(DK)

    def wcols(w):
        # [DM, d] -> [128, NMC*d] (chunk-major columns)
        return np.asarray(w).reshape(NMC, 128, -1).transpose(1, 0, 2).reshape(128, -1)

    w_pack = np.zeros((128, WCOLS), np.float32)
    w_pack[:, WCOL_V:WCOL_V + DM] = wcols(np.asarray(wv, np.float32))
    w_pack[:, WCOL_BQ] = np.asarray(bq, np.float32)
    w_pack[:, WCOL_ONES] = 1.0
    w_pack = w_pack.astype(NP_BF16)
    # Q/K weights in fp8 e4m3 (unscaled), DoubleRow block layout
    # [p, qk, c, j, d]: dm = 256c + 128j + p
    def w8_pack(w):
        return np.asarray(w, np.float32).reshape(NMC // 2, 2, 128, DK) \
            .transpose(2, 0, 1, 3)
    w8 = np.ascontiguousarray(
        np.stack([w8_pack(wq), w8_pack(wk)], axis=1)).astype(NP_FP8)

    in_maps = []
    for core in range(N_CORES):
        b, h = core // 2, core % 2
        q5 = np.asarray(q[b], np.float32).T[:, h * SQ:(h + 1) * SQ] \
            .astype(NP_FP8).reshape(NMC // 2, 2, 128, NQB, 512)
        qTb = np.ascontiguousarray(q5.transpose(3, 2, 0, 4, 1))
        k5 = np.asarray(k[b], np.float32).T.astype(NP_FP8).reshape(
            NMC // 2, 2, 128, NKB, 512)          # [c, j, p, kb, s]
        kTb = np.ascontiguousarray(k5.transpose(3, 2, 0, 4, 1))  # [kb,p,c,s,j]
        vTb = _pack_x(np.asarray(v[b], np.float32).T.astype(NP_BF16), NKB)
        in_maps.append({
            "qT": qTb, "kT": kTb, "vT": vTb, "w_pack": w_pack, "w8": w8,
        })
    return in_maps


def kernel(q, k, v, wq, bq, wk, bk, wv, bv, _trace=False, _tmpdir=None):
    nc = _build()
    in_maps = make_in_maps(q, k, v, wq, bq, wk, bk, wv, bv)
    res = run_bass_kernel_spmd(
        nc, in_maps, list(range(N_CORES)), trace=_trace, tmpdir=_tmpdir
    )
    bv_f = np.asarray(bv, np.float32)
    out = np.empty((B, S, DV), np.float32)
    for core in range(N_CORES):
        b, h = core // 2, core % 2
        r = res.results[core]
        out[b, h * SQ:(h + 1) * SQ, :] = (
            np.asarray(r["outT"], np.float32) / r["sums"]).T + bv_f
    if _trace:
        kernel.last_results = res
    return out


# revision 35
# speedup vs baseline: 1.0799x; 1.0799x over previous
"""Bass/Trainium2 kernel for batched dot-product attention.

Problem: q,k,v [B=4, S=4096, D=1024]; projections to dk=dv=128; softmax
attention per batch element.  Sharded over 8 NeuronCores as (batch,
query-half): core c handles batch c//2, queries (c%2)*2048 ... +2048.

All layouts on-chip keep the contraction dimension on SBUF partitions:
  qT/kT     [d_model, seq]    (host pre-transposed, fp8 pair-interleaved)
  vT        [d_model, seq]    (host pre-transposed, bf16)
  kpT/qpT   [dk, seq]         (projection output, bf16)
  vp        [seq, dv]         (via DMA crossbar transpose, bf16)
  S^T tiles [keys, q]         (scores transposed, f32 PSUM)
  out^T     [dv, q]           (bf16; host divides by sums and undoes)

Q/K projections run as fp8e4 DoubleRow matmuls (contraction 256/step,
2 fp8 per cycle -- requires the moving operand pair-interleaved in
memory and the stationary in [p, 2, d] block layout).  wq stays
UNSCALED so it avoids fp8 subnormals; the 1/sqrt(dk) folds into the
exp's scale operand instead.  V stays bf16 for accuracy.

Query blocks are processed in PAIRS (1024-wide exp tiles amortize the
ScalarE per-op overhead and halve AV weight loads).  Pair 0's attention
chunks interleave with the projection kb-loop; pair 1 defers its AV
matmuls (exp tiles parked in SBUF) until PSUM banks free up, bursting
per query-half at the end so output copies overlap remaining matmuls.
vp transposes ride the sync-queue DMA crossbar, pipelined two blocks
ahead (psv+cast one iteration before the transpose issue) so the sync
engine never stalls the input stream.  Softmax denominators (sum over
keys = partition axis) via a ones-vector matmul over a bf16 pairwise
accumulation tree; normalization on the host.

Bias algebra: bk drops entirely (a per-query score shift is softmax-
invariant); bv is applied on the host (softmax weights sum to 1); bq
adds to qpT on-chip.  Weights ship as one packed bf16 tensor (wv, bq,
ones) plus one fp8 tensor (wq, wk), fetched first on the sync queue.
A burst of dummy matmuls on scratch SBUF warms the PE clock (HAM)
before real data lands.
"""

import math

import numpy as np
import ml_dtypes

import concourse.bass as bass
import concourse.tile as tile
from concourse import bacc, mybir
from concourse.bass_utils import run_bass_kernel_spmd

B, S, DM, DK, DV = 4, 4096, 1024, 128, 128
N_CORES = 8
SQ = S // 2          # queries per core
NQB = SQ // 512      # query blocks of 512 per core (4)
NKC = S // 128       # key chunks of 128 (32)
NMC = DM // 128      # d_model chunks (8)
NKB = S // 512       # key blocks of 512 (8)

BF16 = mybir.dt.bfloat16
F32 = mybir.dt.float32
FP8 = mybir.dt.float8e4
NP_BF16 = ml_dtypes.bfloat16
NP_FP8 = ml_dtypes.float8_e4m3
DoubleRow = mybir.MatmulPerfMode.DoubleRow

E_DT = BF16          # dtype of exp tiles (AV moving operand)
ACC_DT = BF16        # dtype of the denominator accumulation tree
AV_STAGGER = 2       # pair-chunks the exp/AV drain lags the S matmuls
N_WARMUP = 10        # dummy matmuls to keep the PE HAM-warm at startup

Identity = mybir.ActivationFunctionType.Identity
Copy = mybir.ActivationFunctionType.Copy
Exp = mybir.ActivationFunctionType.Exp

# packed weight layout: columns [wv | bq | ones]; wq/wk ship as fp8
WCOL_V = 0
WCOL_BQ, WCOL_ONES = DM, DM + 1
WCOLS = DM + 2
SCALE = 1.0 / math.sqrt(DK)


def _emit(tc: tile.TileContext, aps: dict):
    nc = tc.nc
    qT, kT, vT = aps["qT"], aps["kT"], aps["vT"]
    outT = aps["outT"]

    with tc.tile_pool(name="persist", bufs=1) as persist:
        # --- packed constants (one DMA, first on the sync queue) ---
        w_sb = persist.tile([128, WCOLS], BF16, tag="w_pack", name="w_pack")
        nc.sync.dma_start(w_sb[:], aps["w_pack"][:])

        def wslice(base, c):
            return w_sb[:, base + c * 128: base + (c + 1) * 128]

        # w8[:, 0] = wq (unscaled; 1/sqrt(dk) folds into the exp scale),
        # w8[:, 1] = wk; DoubleRow block layout [p, qk, c, j, d]
        w8_sb = persist.tile([128, 2, NMC // 2, 2, 128], FP8, tag="w8", name="w8")
        ones_ap = w_sb[:, WCOL_ONES:WCOL_ONES + 1]
        bq_f32 = persist.tile([128, 1], F32, tag="bq_f32", name="bq_f32")
        nc.vector.tensor_copy(bq_f32[:], w_sb[:, WCOL_BQ:WCOL_BQ + 1])
        bq_ap = bq_f32[:]

        # --- PE warm-up scratch (no data deps; HAM warms before real MMs) ---
        warm_sb = persist.tile([128, 512], BF16, tag="warm", name="warm_sb")

        # --- persistent activations ---
        kpT_blk = [persist.tile([128, 512], BF16, tag=f"kpT{i}", name=f"kpT{i}")
                   for i in range(NKB)]
        qpT_t = [persist.tile([128, 512], BF16, tag=f"qpT{i}", name=f"qpT{i}")
                 for i in range(NQB)]
        # vp per key-block: vp_blk[kb][p, j, :] = projected V row for key
        # 512*kb + 128*j + p (one xbar transpose per block)
        vp_blk = [persist.tile([128, 4, 128], BF16, tag=f"vpb{i}", name=f"vpb{i}")
                  for i in range(NKB)]
        sums_sb = persist.tile([1, SQ], F32, tag="sums", name="sums_sb")

        with (
            tc.tile_pool(name="op", bufs=2, space="PSUM") as op,
            tc.tile_pool(name="pp", bufs=2, space="PSUM") as pp,
            tc.tile_pool(name="sp", bufs=2, space="PSUM") as sp,
            tc.tile_pool(name="xs", bufs=2) as xs,
            tc.tile_pool(name="ep", bufs=6) as ep,
            tc.tile_pool(name="tp", bufs=3) as tp,
            tc.tile_pool(name="e1p", bufs=1) as e1p,
            tc.tile_pool(name="accp", bufs=2) as accp,
            tc.tile_pool(name="miscp", bufs=2) as miscp,
        ):
            # ---- PE warm-up: dummy matmuls on scratch, discarded ----
            nc.gpsimd.memset(warm_sb[:], 0.0)
            warm_ps = sp.tile([128, 1024], F32, tag="sp", name="warm_ps")
            for _ in range(N_WARMUP):
                nc.tensor.matmul(warm_ps[:, 0:512], lhsT=warm_sb[:, 0:128],
                                 rhs=warm_sb[:], start=True, stop=True)
                nc.tensor.matmul(warm_ps[:, 512:1024], lhsT=warm_sb[:, 0:128],
                                 rhs=warm_sb[:], start=True, stop=True)

            # ---- input fetch + qp projection helpers ----
            kxs, vxs = {}, {}

            def fetch_kx(kb):
                # [p, c, s, j]: dm = 256c + 128j + p, pair elements adjacent
                # so the DoubleRow matmul streams 2 fp8 per cycle
                kx = xs.tile([128, NMC // 2, 512, 2], FP8, tag="kx",
                             name=f"kx{kb}", bufs=4)
                nc.sync.dma_start(kx[:], kT[kb])
                kxs[kb] = kx

            def fetch_vx(kb):
                vx = xs.tile([128, NMC, 512], BF16, tag="vx", name=f"vx{kb}",
                             bufs=4)
                nc.sync.dma_start(vx[:], vT[kb])
                vxs[kb] = vx

            qxs = {}

            def fetch_q(qb):
                qx = xs.tile([128, NMC // 2, 512, 2], FP8, tag="qx",
                             name=f"qx{qb}", bufs=4)
                nc.sync.dma_start(qx[:], qT[qb])
                qxs[qb] = qx

            def project_q(qb):
                qx = qxs.pop(qb)
                psq = sp.tile([128, 512], F32, tag="sp", name=f"psq{qb}")
                for c in range(NMC // 2):
                    nc.tensor.matmul(
                        psq[:], lhsT=w8_sb[:, 0, c, :, :],
                        rhs=qx[:, c, :, :].rearrange("p n j -> p j n"),
                        start=(c == 0), stop=(c == NMC // 2 - 1),
                        perf_mode=DoubleRow,
                    )
                nc.vector.tensor_scalar_add(qpT_t[qb][:], psq[:], bq_ap)

            # stream order: weights already queued first; then the tensors
            # needed to unlock pair-0 attention (qx0, kx0, qx1), then vx0
            # for the AV chain, then the rest.
            nc.sync.dma_start(w8_sb[:], aps["w8"][:])
            fetch_q(0)
            fetch_q(1)
            fetch_kx(0)
            fetch_kx(1)
            fetch_vx(0)
            fetch_q(2)
            fetch_q(3)
            fetch_vx(1)

            def proj_k(kb):
                kx = kxs.pop(kb)
                psk = pp.tile([128, 512], F32, tag="pp", name=f"psk{kb}")
                for c in range(NMC // 2):
                    nc.tensor.matmul(
                        psk[:], lhsT=w8_sb[:, 1, c, :, :],
                        rhs=kx[:, c, :, :].rearrange("p n j -> p j n"),
                        start=(c == 0), stop=(c == NMC // 2 - 1),
                        perf_mode=DoubleRow,
                    )
                nc.vector.tensor_copy(kpT_blk[kb][:], psk[:])

            # ---- attention pair machinery ----
            def pair_begin(pidx, spool, defer_av=False):
                qa, qb_ = 2 * pidx, 2 * pidx + 1
                return dict(
                    p=pidx, qs=(qa, qb_), sp=spool, defer=defer_av,
                    o=None if defer_av else
                      [op.tile([128, 512], F32, tag="op", name=f"o{q}")
                       for q in (qa, qb_)],
                    acc=accp.tile([128, 1024], ACC_DT, tag="acc",
                                  name=f"acc{pidx}"),
                    pend=[], evs=[],
                )

            def pair_drain(st):
                kc, s = st["pend"].pop(0)
                if st["defer"]:
                    e = e1p.tile([128, 1024], E_DT, tag=f"e{st['p']}d_{kc}",
                                 name=f"e{st['p']}_{kc}")
                else:
                    e = ep.tile([128, 1024], E_DT, tag="e", name=f"e{st['p']}_{kc}")
                nc.scalar.activation(e[:], s[:], Exp, scale=SCALE)
                if kc % 2 == 0:
                    st["elast"] = e
                else:
                    # one bf16 add level halves the accumulate traffic; the
                    # acc tree stays bf16 for DVE 2x mode.  The serial
                    # acc-chain alternates onto GpSimd so DVE stays free for
                    # the projection casts (PSUM recycling path).
                    tmp = tp.tile([128, 1024], ACC_DT, tag="tmp",
                                  name=f"t{st['p']}_{kc}")
                    nc.vector.tensor_add(tmp[:], st["elast"][:], e[:])
                    if kc == 1:
                        nc.vector.tensor_copy(st["acc"][:], tmp[:])
                    else:
                        nc.vector.tensor_add(st["acc"][:], st["acc"][:], tmp[:])
                if st["defer"]:
                    st["evs"].append((kc, e))
                    return
                av_emit(st, kc, e)

            def av_emit(st, kc, e):
                vps = vp_blk[kc // 4][:, kc % 4, :]
                for h in range(2):
                    nc.tensor.matmul(
                        st["o"][h][:], lhsT=vps, rhs=e[:, h * 512:(h + 1) * 512],
                        start=(kc == 0), stop=(kc == NKC - 1),
                    )

            def pair_chunk(st, kc):
                s = st["sp"].tile([128, 1024], F32, tag="sp", name=f"s{st['p']}_{kc}")
                kslice = kpT_blk[kc // 4][:, (kc % 4) * 128:(kc % 4 + 1) * 128]
                for h in range(2):
                    nc.tensor.matmul(
                        s[:, h * 512:(h + 1) * 512], lhsT=kslice,
                        rhs=qpT_t[st["qs"][h]][:], start=True, stop=True,
                    )
                st["pend"].append((kc, s))
                if len(st["pend"]) > AV_STAGGER:
                    pair_drain(st)

            def pair_flush(st):
                while st["pend"]:
                    pair_drain(st)
                if st["defer"]:
                    # use the projection pool's banks (free after the kb
                    # loop) so the burst need not wait for pair 0's output
                    # copies to release the op slots
                    st["o"] = [pp.tile([128, 512], F32, tag="pp", name=f"o{q}")
                               for q in st["qs"]]
                    for kc, e in st["evs"]:
                        av_emit(st, kc, e)

            def pair_tail_half(st, h):
                q = st["qs"][h]
                ps_sum = st["sp"].tile([1, 512], F32, tag="sp", name=f"pssum{q}")
                nc.tensor.matmul(
                    ps_sum[:], lhsT=ones_ap,
                    rhs=st["acc"][:, h * 512:(h + 1) * 512],
                    start=True, stop=True,
                )
                nc.scalar.activation(
                    sums_sb[:, q * 512:(q + 1) * 512], ps_sum[:], Copy
                )
                outsb = miscp.tile([128, 512], BF16, tag="out", name=f"out{q}")
                nc.vector.tensor_copy(outsb[:], st["o"][h][:])
                nc.sync.dma_start(outT[:, q * 512:(q + 1) * 512], outsb[:])

            def pair_tail(st):
                for h in range(2):
                    pair_tail_half(st, h)

            vpts = {}

            def proj_v_mm(kb):
                # V-projection matmuls + PSUM->SBUF cast only; the xbar
                # transpose is issued separately, one iteration later, so
                # it never blocks the sync queue waiting on vpt
                vx = vxs.pop(kb)
                psv = pp.tile([128, 512], F32, tag="pp", name=f"psv{kb}")
                for c in range(NMC):
                    nc.tensor.matmul(
                        psv[:], lhsT=wslice(WCOL_V, c), rhs=vx[:, c, :],
                        start=(c == 0), stop=(c == NMC - 1),
                    )
                vpt = xs.tile([128, 512], BF16, tag="vpt", name=f"vpt{kb}",
                              bufs=3)
                nc.vector.tensor_copy(vpt[:], psv[:])
                vpts[kb] = vpt

            def proj_v_tr(kb):
                nc.sync.dma_start_transpose(vp_blk[kb][:], vpts.pop(kb)[:])

            # ---- kb loop: kp + vp projection, pair-0 attention interleaved ----
            st0 = pair_begin(0, sp)
            st1 = pair_begin(1, sp, defer_av=True)
            project_q(0)
            project_q(1)
            proj_k(0)
            for kb in range(NKB):
                if kb + 2 < NKB:
                    fetch_kx(kb + 2)
                    fetch_vx(kb + 2)

                if kb == 0:
                    # vp0 is not ready until vx0 lands, so pair 0's first
                    # AVs are deferred (exp-only) and burst in iteration 1
                    st0["defer"] = True
                    pair_chunk(st0, 0)
                    pair_chunk(st0, 1)
                    proj_k(1)
                    pair_chunk(st0, 2)
                    pair_chunk(st0, 3)
                    # drain fully: st1's S matmuls below recycle these PSUM
                    # slots, and their release must not depend on scalar-queue
                    # work emitted after st1's exps (circular wait)
                    while st0["pend"]:
                        pair_drain(st0)
                    proj_v_mm(0)
                    proj_v_tr(0)
                    project_q(2)
                    project_q(3)
                    for kc in range(4):
                        pair_chunk(st1, kc)
                    proj_v_mm(1)
                    proj_v_tr(1)
                    proj_v_mm(2)
                else:
                    # projections first: they have no PSUM/exp coupling, so
                    # the PE works while ScalarE catches up on the exp chain
                    if kb + 1 < NKB:
                        proj_k(kb + 1)
                        # transpose for block kb+1: its vpt was cast last
                        # iteration, so the sync engine never waits on it
                        proj_v_tr(kb + 1)
                    if kb == 1:
                        st0["defer"] = False
                        for kc_, e_ in st0["evs"]:
                            av_emit(st0, kc_, e_)
                        st0["evs"] = []
                    kc = 4 * kb
                    pair_chunk(st0, kc)
                    pair_chunk(st1, kc)
                    pair_chunk(st0, kc + 1)
                    pair_chunk(st1, kc + 1)
                    pair_chunk(st0, kc + 2)
                    pair_chunk(st1, kc + 2)
                    if kb + 2 < NKB:
                        proj_v_mm(kb + 2)
                    pair_chunk(st0, kc + 3)
                    pair_chunk(st1, kc + 3)
            # early AV burst: everything already exp'd can run on the PE
            # while ScalarE finishes the exp-chain tail
            st1["o"] = [pp.tile([128, 512], F32, tag="pp", name=f"o{q}")
                        for q in st1["qs"]]
            for kc, e in st1["evs"]:
                av_emit(st1, kc, e)
            st1["evs"] = []
            pair_flush(st0)
            pair_tail(st0)
            while st1["pend"]:
                pair_drain(st1)
            # final chunks per half: half 0 finishes first so its output
            # copy + DMA overlap half 1's remaining AV matmuls
            for h in range(2):
                for kc, e in st1["evs"]:
                    nc.tensor.matmul(
                        st1["o"][h][:], lhsT=vp_blk[kc // 4][:, kc % 4, :],
                        rhs=e[:, h * 512:(h + 1) * 512],
                        start=(kc == 0), stop=(kc == NKC - 1),
                    )
                pair_tail_half(st1, h)
            nc.sync.dma_start(aps["sums"][:], sums_sb[:])


_CACHE = {}


def _build():
    if "nc" in _CACHE:
        return _CACHE["nc"]
    nc = bacc.Bacc("TRN2", debug=False, num_devices=N_CORES)
    aps = {
        "qT": nc.dram_tensor("qT", [NQB, 128, NMC // 2, 512, 2], FP8,
                             kind="ExternalInput").ap(),
        "kT": nc.dram_tensor("kT", [NKB, 128, NMC // 2, 512, 2], FP8,
                             kind="ExternalInput").ap(),
        "vT": nc.dram_tensor("vT", [NKB, 128, NMC, 512], BF16,
                             kind="ExternalInput").ap(),
        "w_pack": nc.dram_tensor("w_pack", [128, WCOLS], BF16,
                                 kind="ExternalInput").ap(),
        "w8": nc.dram_tensor("w8", [128, 2, NMC // 2, 2, 128], FP8,
                             kind="ExternalInput").ap(),
        "outT": nc.dram_tensor("outT", [DV, SQ], BF16, kind="ExternalOutput").ap(),
        "sums": nc.dram_tensor("sums", [1, SQ], F32, kind="ExternalOutput").ap(),
    }
    with tile.TileContext(nc) as tc:
        _emit(tc, aps)
    nc.compile()
    _CACHE["nc"] = nc
    return nc


def _pack_x(xT, nblk):
    # [DM, n] -> [nblk, 128, NMC, 512]  (contiguous per-stripe layout)
    return np.ascontiguousarray(
        xT.reshape(NMC, 128, nblk, 512).transpose(2, 1, 0, 3))


def make_in_maps(q, k, v, wq, bq, wk, bk, wv, bv):
    scale = 1.0 / math.sqrt(DK)

    def wcols(w):
        # [DM, d] -> [128, NMC*d] (chunk-major columns)
        return np.asarray(w).reshape(NMC, 128, -1).transpose(1, 0, 2).reshape(128, -1)

    w_pack = np.zeros((128, WCOLS), np.float32)
    w_pack[:, WCOL_V:WCOL_V + DM] = wcols(np.asarray(wv, np.float32))
    w_pack[:, WCOL_BQ] = np.asarray(bq, np.float32)
    w_pack[:, WCOL_ONES] = 1.0
    w_pack = w_pack.astype(NP_BF16)
    # Q/K weights in fp8 e4m3 (unscaled), DoubleRow block layout
    # [p, qk, c, j, d]: dm = 256c + 128j + p
    def w8_pack(w):
        return np.asarray(w, np.float32).reshape(NMC // 2, 2, 128, DK) \
            .transpose(2, 0, 1, 3)
    w8 = np.ascontiguousarray(
        np.stack([w8_pack(wq), w8_pack(wk)], axis=1)).astype(NP_FP8)

    in_maps = []
    for core in range(N_CORES):
        b, h = core // 2, core % 2
        q5 = np.asarray(q[b], np.float32).T[:, h * SQ:(h + 1) * SQ] \
            .astype(NP_FP8).reshape(NMC // 2, 2, 128, NQB, 512)
        qTb = np.ascontiguousarray(q5.transpose(3, 2, 0, 4, 1))
        k5 = np.asarray(k[b], np.float32).T.astype(NP_FP8).reshape(
            NMC // 2, 2, 128, NKB, 512)          # [c, j, p, kb, s]
        kTb = np.ascontiguousarray(k5.transpose(3, 2, 0, 4, 1))  # [kb,p,c,s,j]
        vTb = _pack_x(np.asarray(v[b], np.float32).T.astype(NP_BF16), NKB)
        in_maps.append({
            "qT": qTb, "kT": kTb, "vT": vTb, "w_pack": w_pack, "w8": w8,
        })
    return in_maps


def kernel(q, k, v, wq, bq, wk, bk, wv, bv, _trace=False, _tmpdir=None):
    nc = _build()
    in_maps = make_in_maps(q, k, v, wq, bq, wk, bk, wv, bv)
    res = run_bass_kernel_spmd(
        nc, in_maps, list(range(N_CORES)), trace=_trace, tmpdir=_tmpdir
    )
    bv_f = np.asarray(bv, np.float32)
    out = np.empty((B, S, DV), np.float32)
    for core in range(N_CORES):
        b, h = core // 2, core % 2
        r = res.results[core]
        out[b, h * SQ:(h + 1) * SQ, :] = (
            np.asarray(r["outT"], np.float32) / r["sums"]).T + bv_f
    if _trace:
        kernel.last_results = res
    return out


# revision 36
# speedup vs baseline: 1.2725x; 1.1783x over previous
"""Bass/Trainium2 kernel for batched dot-product attention.

Problem: q,k,v [B=4, S=4096, D=1024]; projections to dk=dv=128; softmax
attention per batch element.  Sharded over 8 NeuronCores as (batch,
query-half): core c handles batch c//2, queries (c%2)*2048 ... +2048.

All layouts on-chip keep the contraction dimension on SBUF partitions:
  qT/kT     [d_model, seq]    (host pre-transposed, fp8 pair-interleaved)
  vT        [d_model, seq]    (host pre-transposed, bf16)
  kpT/qpT   [dk, seq]         (projection output, bf16)
  vp        [seq, dv]         (via DMA crossbar transpose, bf16)
  S^T tiles [keys, q]         (scores transposed, f32 PSUM)
  out^T     [dv, q]           (bf16; host divides by sums and undoes)

Q/K projections run as fp8e4 DoubleRow matmuls (contraction 256/step,
2 fp8 per cycle -- requires the moving operand pair-interleaved in
memory and the stationary in [p, 2, d] block layout).  wq stays
UNSCALED so it avoids fp8 subnormals; the 1/sqrt(dk) folds into the
exp's scale operand instead.  V stays bf16 for accuracy.

Query blocks are processed in PAIRS (1024-wide exp tiles amortize the
ScalarE per-op overhead and halve AV weight loads).  Pair 0's attention
chunks interleave with the projection kb-loop; pair 1 defers its AV
matmuls (exp tiles parked in SBUF) until PSUM banks free up, bursting
per query-half at the end so output copies overlap remaining matmuls.
vp transposes ride the sync-queue DMA crossbar, pipelined two blocks
ahead (psv+cast one iteration before the transpose issue) so the sync
engine never stalls the input stream.  Softmax denominators (sum over
keys = partition axis) via a ones-vector matmul over a bf16 pairwise
accumulation tree; normalization on the host.

Bias algebra: bk drops entirely (a per-query score shift is softmax-
invariant); bv is applied on the host (softmax weights sum to 1); bq
adds to qpT on-chip.  Weights ship as one packed bf16 tensor (wv, bq,
ones) plus one fp8 tensor (wq, wk), fetched first on the sync queue.
A burst of dummy matmuls on scratch SBUF warms the PE clock (HAM)
before real data lands.
"""

import math

import numpy as np
import ml_dtypes

import concourse.bass as bass
import concourse.tile as tile
from concourse import bacc, mybir
from concourse.bass_utils import run_bass_kernel_spmd

B, S, DM, DK, DV = 4, 4096, 1024, 128, 128
N_CORES = 8
SQ = S // 2          # queries per core
NQB = SQ // 512      # query blocks of 512 per core (4)
NKC = S // 128       # key chunks of 128 (32)
NMC = DM // 128      # d_model chunks (8)
NKB = S // 512       # key blocks of 512 (8)

BF16 = mybir.dt.bfloat16
F32 = mybir.dt.float32
FP8 = mybir.dt.float8e4
NP_BF16 = ml_dtypes.bfloat16
NP_FP8 = ml_dtypes.float8_e4m3
DoubleRow = mybir.MatmulPerfMode.DoubleRow

E_DT = BF16          # dtype of exp tiles (AV moving operand)
ACC_DT = BF16        # dtype of the denominator accumulation tree
AV_STAGGER = 2       # pair-chunks the exp/AV drain lags the S matmuls
N_WARMUP = 10        # dummy matmuls to keep the PE HAM-warm at startup

Identity = mybir.ActivationFunctionType.Identity
Copy = mybir.ActivationFunctionType.Copy
Exp = mybir.ActivationFunctionType.Exp

# packed weight layout: columns [wv | bq | ones]; wq/wk ship as fp8
WCOL_V = 0
WCOL_BQ, WCOL_ONES = DM, DM + 1
WCOLS = DM + 2
SCALE = 1.0 / math.sqrt(DK)


def _emit(tc: tile.TileContext, aps: dict):
    nc = tc.nc
    qT, kT, vT = aps["qT"], aps["kT"], aps["vT"]
    outT = aps["outT"]

    with tc.tile_pool(name="persist", bufs=1) as persist:
        # --- packed constants (one DMA, first on the sync queue) ---
        w_sb = persist.tile([128, WCOLS], BF16, tag="w_pack", name="w_pack")
        nc.sync.dma_start(w_sb[:], aps["w_pack"][:])

        def wslice(base, c):
            return w_sb[:, base + c * 128: base + (c + 1) * 128]

        # w8[:, 0] = wq (unscaled; 1/sqrt(dk) folds into the exp scale),
        # w8[:, 1] = wk; DoubleRow block layout [p, qk, c, j, d]
        w8_sb = persist.tile([128, 2, NMC // 2, 2, 128], FP8, tag="w8", name="w8")
        ones_ap = w_sb[:, WCOL_ONES:WCOL_ONES + 1]
        bq_f32 = persist.tile([128, 1], F32, tag="bq_f32", name="bq_f32")
        nc.vector.tensor_copy(bq_f32[:], w_sb[:, WCOL_BQ:WCOL_BQ + 1])
        bq_ap = bq_f32[:]

        # --- PE warm-up scratch (no data deps; HAM warms before real MMs) ---
        warm_sb = persist.tile([128, 512], BF16, tag="warm", name="warm_sb")

        # --- persistent activations ---
        kpT_blk = [persist.tile([128, 512], BF16, tag=f"kpT{i}", name=f"kpT{i}")
                   for i in range(NKB)]
        qpT_t = [persist.tile([128, 512], BF16, tag=f"qpT{i}", name=f"qpT{i}")
                 for i in range(NQB)]
        # vp per key-block: vp_blk[kb][p, j, :] = projected V row for key
        # 512*kb + 128*j + p (one xbar transpose per block)
        vp_blk = [persist.tile([128, 4, 128], BF16, tag=f"vpb{i}", name=f"vpb{i}")
                  for i in range(NKB)]
        sums_sb = persist.tile([1, SQ], F32, tag="sums", name="sums_sb")

        with (
            tc.tile_pool(name="op", bufs=2, space="PSUM") as op,
            tc.tile_pool(name="pp", bufs=2, space="PSUM") as pp,
            tc.tile_pool(name="sp", bufs=2, space="PSUM") as sp,
            tc.tile_pool(name="xs", bufs=2) as xs,
            tc.tile_pool(name="ep", bufs=6) as ep,
            tc.tile_pool(name="tp", bufs=3) as tp,
            tc.tile_pool(name="e1p", bufs=1) as e1p,
            tc.tile_pool(name="accp", bufs=2) as accp,
            tc.tile_pool(name="miscp", bufs=2) as miscp,
        ):
            # ---- PE warm-up: dummy matmuls on scratch, discarded ----
            nc.gpsimd.memset(warm_sb[:], 0.0)
            warm_ps = sp.tile([128, 1024], F32, tag="sp", name="warm_ps")
            for _ in range(N_WARMUP):
                nc.tensor.matmul(warm_ps[:, 0:512], lhsT=warm_sb[:, 0:128],
                                 rhs=warm_sb[:], start=True, stop=True)
                nc.tensor.matmul(warm_ps[:, 512:1024], lhsT=warm_sb[:, 0:128],
                                 rhs=warm_sb[:], start=True, stop=True)

            # ---- input fetch + qp projection helpers ----
            kxs, vxs = {}, {}

            def fetch_kx(kb):
                # [p, c, s, j]: dm = 256c + 128j + p, pair elements adjacent
                # so the DoubleRow matmul streams 2 fp8 per cycle
                kx = xs.tile([128, NMC // 2, 512, 2], FP8, tag="kx",
                             name=f"kx{kb}", bufs=4)
                nc.sync.dma_start(kx[:], kT[kb])
                kxs[kb] = kx

            def fetch_vx(kb):
                vx = xs.tile([128, NMC, 512], BF16, tag="vx", name=f"vx{kb}",
                             bufs=4)
                nc.sync.dma_start(vx[:], vT[kb])
                vxs[kb] = vx

            qxs = {}

            def fetch_q(qb):
                qx = xs.tile([128, NMC // 2, 512, 2], FP8, tag="qx",
                             name=f"qx{qb}", bufs=4)
                nc.sync.dma_start(qx[:], qT[qb])
                qxs[qb] = qx

            def project_q(qb):
                qx = qxs.pop(qb)
                psq = sp.tile([128, 512], F32, tag="sp", name=f"psq{qb}")
                for c in range(NMC // 2):
                    nc.tensor.matmul(
                        psq[:], lhsT=w8_sb[:, 0, c, :, :],
                        rhs=qx[:, c, :, :].rearrange("p n j -> p j n"),
                        start=(c == 0), stop=(c == NMC // 2 - 1),
                        perf_mode=DoubleRow,
                    )
                nc.vector.tensor_scalar_add(qpT_t[qb][:], psq[:], bq_ap)

            # stream order: weights already queued first; then the tensors
            # needed to unlock pair-0 attention (qx0, kx0, qx1), then vx0
            # for the AV chain, then the rest.
            nc.sync.dma_start(w8_sb[:], aps["w8"][:])
            fetch_q(0)
            fetch_q(1)
            fetch_kx(0)
            fetch_kx(1)
            fetch_vx(0)
            fetch_q(2)
            fetch_q(3)
            fetch_vx(1)

            def proj_k(kb):
                kx = kxs.pop(kb)
                psk = pp.tile([128, 512], F32, tag="pp", name=f"psk{kb}")
                for c in range(NMC // 2):
                    nc.tensor.matmul(
                        psk[:], lhsT=w8_sb[:, 1, c, :, :],
                        rhs=kx[:, c, :, :].rearrange("p n j -> p j n"),
                        start=(c == 0), stop=(c == NMC // 2 - 1),
                        perf_mode=DoubleRow,
                    )
                nc.vector.tensor_copy(kpT_blk[kb][:], psk[:])

            # ---- attention pair machinery ----
            def pair_begin(pidx, spool, defer_av=False):
                qa, qb_ = 2 * pidx, 2 * pidx + 1
                return dict(
                    p=pidx, qs=(qa, qb_), sp=spool, defer=defer_av,
                    o=None if defer_av else
                      [op.tile([128, 512], F32, tag="op", name=f"o{q}")
                       for q in (qa, qb_)],
                    acc=accp.tile([128, 1024], ACC_DT, tag="acc",
                                  name=f"acc{pidx}"),
                    pend=[], evs=[],
                )

            def pair_drain(st):
                kc, s = st["pend"].pop(0)
                if st["defer"]:
                    e = e1p.tile([128, 1024], E_DT, tag=f"e{st['p']}d_{kc}",
                                 name=f"e{st['p']}_{kc}")
                else:
                    e = ep.tile([128, 1024], E_DT, tag="e", name=f"e{st['p']}_{kc}")
                nc.scalar.activation(e[:], s[:], Exp, scale=SCALE)
                if kc % 2 == 0:
                    st["elast"] = e
                else:
                    # one bf16 add level halves the accumulate traffic; the
                    # acc tree stays bf16 for DVE 2x mode.  The serial
                    # acc-chain alternates onto GpSimd so DVE stays free for
                    # the projection casts (PSUM recycling path).
                    tmp = tp.tile([128, 1024], ACC_DT, tag="tmp",
                                  name=f"t{st['p']}_{kc}")
                    nc.vector.tensor_add(tmp[:], st["elast"][:], e[:])
                    if kc == 1:
                        nc.vector.tensor_copy(st["acc"][:], tmp[:])
                    else:
                        nc.vector.tensor_add(st["acc"][:], st["acc"][:], tmp[:])
                if st["defer"]:
                    st["evs"].append((kc, e))
                    return
                av_emit(st, kc, e)

            def av_emit(st, kc, e):
                vps = vp_blk[kc // 4][:, kc % 4, :]
                for h in range(2):
                    nc.tensor.matmul(
                        st["o"][h][:], lhsT=vps, rhs=e[:, h * 512:(h + 1) * 512],
                        start=(kc == 0), stop=(kc == NKC - 1),
                    )

            def pair_chunk(st, kc):
                s = st["sp"].tile([128, 1024], F32, tag="sp", name=f"s{st['p']}_{kc}")
                kslice = kpT_blk[kc // 4][:, (kc % 4) * 128:(kc % 4 + 1) * 128]
                for h in range(2):
                    nc.tensor.matmul(
                        s[:, h * 512:(h + 1) * 512], lhsT=kslice,
                        rhs=qpT_t[st["qs"][h]][:], start=True, stop=True,
                    )
                st["pend"].append((kc, s))
                if len(st["pend"]) > AV_STAGGER:
                    pair_drain(st)

            def pair_flush(st):
                while st["pend"]:
                    pair_drain(st)
                if st["defer"]:
                    # use the projection pool's banks (free after the kb
                    # loop) so the burst need not wait for pair 0's output
                    # copies to release the op slots
                    st["o"] = [pp.tile([128, 512], F32, tag="pp", name=f"o{q}")
                               for q in st["qs"]]
                    for kc, e in st["evs"]:
                        av_emit(st, kc, e)

            def pair_tail_half(st, h):
                q = st["qs"][h]
                ps_sum = st["sp"].tile([1, 512], F32, tag="sp", name=f"pssum{q}")
                nc.tensor.matmul(
                    ps_sum[:], lhsT=ones_ap,
                    rhs=st["acc"][:, h * 512:(h + 1) * 512],
                    start=True, stop=True,
                )
                nc.scalar.activation(
                    sums_sb[:, q * 512:(q + 1) * 512], ps_sum[:], Copy
                )
                outsb = miscp.tile([128, 512], BF16, tag="out", name=f"out{q}")
                nc.vector.tensor_copy(outsb[:], st["o"][h][:])
                nc.sync.dma_start(outT[:, q * 512:(q + 1) * 512], outsb[:])

            def pair_tail(st):
                for h in range(2):
                    pair_tail_half(st, h)

            def proj_v(kb):
                # direct [keys, dv] orientation: stationary = vx key-slice,
                # moving = wv chunk (N=128, FWL weight loads).  Four 8-matmul
                # accumulation groups share one PSUM bank, then a single DVE
                # copy lands the vp_blk layout -- no crossbar transpose, no
                # sync-queue drain barriers.
                vx = vxs.pop(kb)
                psv = pp.tile([128, 4, 128], F32, tag="pp", name=f"psv{kb}")
                for j in range(4):
                    for c in range(NMC):
                        nc.tensor.matmul(
                            psv[:, j, :],
                            lhsT=vx[:, c, j * 128:(j + 1) * 128],
                            rhs=wslice(WCOL_V, c),
                            start=(c == 0), stop=(c == NMC - 1),
                        )
                nc.vector.tensor_copy(vp_blk[kb][:], psv[:])

            # ---- kb loop: kp + vp projection, pair-0 attention interleaved ----
            st0 = pair_begin(0, sp)
            st1 = pair_begin(1, sp, defer_av=True)
            project_q(0)
            project_q(1)
            proj_k(0)
            for kb in range(NKB):
                if kb + 2 < NKB:
                    fetch_kx(kb + 2)
                    fetch_vx(kb + 2)

                if kb == 0:
                    # vp0 is not ready until vx0 lands, so pair 0's first
                    # AVs are deferred (exp-only) and burst in iteration 1
                    st0["defer"] = True
                    pair_chunk(st0, 0)
                    pair_chunk(st0, 1)
                    proj_k(1)
                    pair_chunk(st0, 2)
                    pair_chunk(st0, 3)
                    # drain fully: st1's S matmuls below recycle these PSUM
                    # slots, and their release must not depend on scalar-queue
                    # work emitted after st1's exps (circular wait)
                    while st0["pend"]:
                        pair_drain(st0)
                    proj_v(0)
                    project_q(2)
                    project_q(3)
                    for kc in range(4):
                        pair_chunk(st1, kc)
                    proj_v(1)
                else:
                    # projections first: they have no PSUM/exp coupling, so
                    # the PE works while ScalarE catches up on the exp chain
                    if kb + 1 < NKB:
                        proj_k(kb + 1)
                    if kb == 1:
                        st0["defer"] = False
                        for kc_, e_ in st0["evs"]:
                            av_emit(st0, kc_, e_)
                        st0["evs"] = []
                    kc = 4 * kb
                    pair_chunk(st0, kc)
                    pair_chunk(st1, kc)
                    pair_chunk(st0, kc + 1)
                    pair_chunk(st1, kc + 1)
                    pair_chunk(st0, kc + 2)
                    pair_chunk(st1, kc + 2)
                    if kb + 1 < NKB:
                        proj_v(kb + 1)
                    pair_chunk(st0, kc + 3)
                    pair_chunk(st1, kc + 3)
            # early AV burst: everything already exp'd can run on the PE
            # while ScalarE finishes the exp-chain tail
            st1["o"] = [pp.tile([128, 512], F32, tag="pp", name=f"o{q}")
                        for q in st1["qs"]]
            for kc, e in st1["evs"]:
                av_emit(st1, kc, e)
            st1["evs"] = []
            pair_flush(st0)
            pair_tail(st0)
            while st1["pend"]:
                pair_drain(st1)
            # final chunks per half: half 0 finishes first so its output
            # copy + DMA overlap half 1's remaining AV matmuls
            for h in range(2):
                for kc, e in st1["evs"]:
                    nc.tensor.matmul(
                        st1["o"][h][:], lhsT=vp_blk[kc // 4][:, kc % 4, :],
                        rhs=e[:, h * 512:(h + 1) * 512],
                        start=(kc == 0), stop=(kc == NKC - 1),
                    )
                pair_tail_half(st1, h)
            nc.sync.dma_start(aps["sums"][:], sums_sb[:])


_CACHE = {}


def _build():
    if "nc" in _CACHE:
        return _CACHE["nc"]
    nc = bacc.Bacc("TRN2", debug=False, num_devices=N_CORES)
    aps = {
        "qT": nc.dram_tensor("qT", [NQB, 128, NMC // 2, 512, 2], FP8,
                             kind="ExternalInput").ap(),
        "kT": nc.dram_tensor("kT", [NKB, 128, NMC // 2, 512, 2], FP8,
                             kind="ExternalInput").ap(),
        "vT": nc.dram_tensor("vT", [NKB, 128, NMC, 512], BF16,
                             kind="ExternalInput").ap(),
        "w_pack": nc.dram_tensor("w_pack", [128, WCOLS], BF16,
                                 kind="ExternalInput").ap(),
        "w8": nc.dram_tensor("w8", [128, 2, NMC // 2, 2, 128], FP8,
                             kind="ExternalInput").ap(),
        "outT": nc.dram_tensor("outT", [DV, SQ], BF16, kind="ExternalOutput").ap(),
        "sums": nc.dram_tensor("sums", [1, SQ], F32, kind="ExternalOutput").ap(),
    }
    with tile.TileContext(nc) as tc:
        _emit(tc, aps)
    nc.compile()
    _CACHE["nc"] = nc
    return nc


def _pack_x(xT, nblk):
    # [DM, n] -> [nblk, 128, NMC, 512]  (contiguous per-stripe layout)
    return np.ascontiguousarray(
        xT.reshape(NMC, 128, nblk, 512).transpose(2, 1, 0, 3))


def make_in_maps(q, k, v, wq, bq, wk, bk, wv, bv):
    scale = 1.0 / math.sqrt(DK)

    def wcols(w):
        # [DM, d] -> [128, NMC*d] (chunk-major columns)
        return np.asarray(w).reshape(NMC, 128, -1).transpose(1, 0, 2).reshape(128, -1)

    w_pack = np.zeros((128, WCOLS), np.float32)
    w_pack[:, WCOL_V:WCOL_V + DM] = wcols(np.asarray(wv, np.float32))
    w_pack[:, WCOL_BQ] = np.asarray(bq, np.float32)
    w_pack[:, WCOL_ONES] = 1.0
    w_pack = w_pack.astype(NP_BF16)
    # Q/K weights in fp8 e4m3 (unscaled), DoubleRow block layout
    # [p, qk, c, j, d]: dm = 256c + 128j + p
    def w8_pack(w):
        return np.asarray(w, np.float32).reshape(NMC // 2, 2, 128, DK) \
            .transpose(2, 0, 1, 3)
    w8 = np.ascontiguousarray(
        np.stack([w8_pack(wq), w8_pack(wk)], axis=1)).astype(NP_FP8)

    in_maps = []
    for core in range(N_CORES):
        b, h = core // 2, core % 2
        q5 = np.asarray(q[b], np.float32).T[:, h * SQ:(h + 1) * SQ] \
            .astype(NP_FP8).reshape(NMC // 2, 2, 128, NQB, 512)
        qTb = np.ascontiguousarray(q5.transpose(3, 2, 0, 4, 1))
        k5 = np.asarray(k[b], np.float32).T.astype(NP_FP8).reshape(
            NMC // 2, 2, 128, NKB, 512)          # [c, j, p, kb, s]
        kTb = np.ascontiguousarray(k5.transpose(3, 2, 0, 4, 1))  # [kb,p,c,s,j]
        vTb = _pack_x(np.asarray(v[b], np.float32).T.astype(NP_BF16), NKB)
        in_maps.append({
            "qT": qTb, "kT": kTb, "vT": vTb, "w_pack": w_pack, "w8": w8,
        })
    return in_maps


def kernel(q, k, v, wq, bq, wk, bk, wv, bv, _trace=False, _tmpdir=None):
    nc = _build()
    in_maps = make_in_maps(q, k, v, wq, bq, wk, bk, wv, bv)
    res = run_bass_kernel_spmd(
        nc, in_maps, list(range(N_CORES)), trace=_trace, tmpdir=_tmpdir
    )
    bv_f = np.asarray(bv, np.float32)
    out = np.empty((B, S, DV), np.float32)
    for core in range(N_CORES):
        b, h = core // 2, core % 2
        r = res.results[core]
        out[b, h * SQ:(h + 1) * SQ, :] = (
            np.asarray(r["outT"], np.float32) / r["sums"]).T + bv_f
    if _trace:
        kernel.last_results = res
    return out
